# revision 11
# baseline (speedup 1.0000x reference)
"""CQAttention Trainium2 kernel.

Math (per batch b, H=256, q=2048, d=8192):
  Qp   = gelu(Q @ W.T + b)                       [q, H]
  S    = C @ Qp.T                                [d, q]
  P    = softmax(S, axis=q)
  out  = P @ Qp + C                              [d, H]

Sharding: data-parallel over batch, one batch per NeuronCore (8 cores).

Per-core pipeline:
  - Loads are f32 HWDGE DMAs split across two sequencers (W/Q-early/bias
    on ACT which is idle at startup; C/Q-late plus all XBAR transposes
    and output stores on SP) with fp16 casts on DVE.
  - Latency-critical transposes at startup (W^T, Q^T groups 0-1, C^T
    chunk 0) run on the PE; the well-pipelined rest (Q^T groups 2-3, C^T
    chunks >= 1) on the DMA XBAR (dma_start_transpose, 14ns/16x128
    tile), keeping the PE free for matmuls.
  - QpT = gelu(W Q^T + b) with per-partition bias on ACT; Qp (natural,
    bf16) by PE-transposing QpT back, with a ones column so the softmax
    denominator falls out of the attended matmul's PSUM accumulation.
  - Per 512-row chunk of C: logits^T tiles [q=128, d=512] with fp16
    operands (bf16 fails the 2e-2 gate); exp on ACT straight from PSUM
    to bf16 (no max-subtraction: |logits| < ~70 so fp32 exp is safe);
    attended accumulated over 16 q-tiles into PSUM [d=128, 257] whose
    column 256 is the row-sum; fused epilogue
    out = (attended * 1/rowsum) + C in one DVE op per tile.
  - Chunk pipeline: C loads 3 chunks ahead, XBAR transposes 2 ahead,
    attended lags logits/exp by 2 q-tiles.
"""

from contextlib import ExitStack

import numpy as np

import concourse.mybir as mybir
import concourse.tile as tile
from concourse import bacc
from concourse.bass_utils import run_bass_kernel_spmd
from concourse.masks import make_identity

B, QL, D, H = 8, 2048, 8192, 256
N_CORES = 8
F32 = mybir.dt.float32
BF16 = mybir.dt.bfloat16
F16 = mybir.dt.float16

HC = H // 128      # feature chunks (2)
NQT = QL // 128    # q tiles (16)
NQG = NQT // 4     # q groups of 4 tiles (4)
DC = 512           # d-chunk size
NDC = D // DC      # d chunks (16)
NDM = DC // 128    # d tiles per chunk (4)

LS = F16  # logits-matmul operand dtype

AF = mybir.ActivationFunctionType
ALU = mybir.AluOpType


def build_body(ctx: ExitStack, tc: tile.TileContext, nc, Qd, Cd, Wd, bd, Od):
    singles = ctx.enter_context(tc.tile_pool(name="singles", bufs=1))
    qstat = ctx.enter_context(tc.tile_pool(name="qstat", bufs=1))
    cpool = ctx.enter_context(tc.tile_pool(name="cpool", bufs=5))
    ctpool = ctx.enter_context(tc.tile_pool(name="ctp", bufs=4))
    exppool = ctx.enter_context(tc.tile_pool(name="expp", bufs=2))
    outpool = ctx.enter_context(tc.tile_pool(name="outp", bufs=3))
    small = ctx.enter_context(tc.tile_pool(name="small", bufs=4))
    psum_l = ctx.enter_context(tc.tile_pool(name="psl", bufs=2, space="PSUM"))
    psum_t = ctx.enter_context(tc.tile_pool(name="pst", bufs=2, space="PSUM"))
    psum_a = ctx.enter_context(tc.tile_pool(name="psa", bufs=1, space="PSUM"))

    ident = singles.tile([128, 128], LS)
    make_identity(nc, ident)

    # Dummy matmuls to ramp the PE out of its p-state throttle (full clock
    # needs ~3us of continuous execution) while the first loads are in
    # flight; results are never read.
    warm = singles.tile([128, 512], LS, name="warm")
    nc.gpsimd.memset(warm[:], 0.0)
    for _ in range(16):
        pw = psum_l.tile([128, 512], F32, tag="pl", name="warmup")
        nc.tensor.matmul(pw[:], ident[:], warm[:], start=True, stop=True)

    # --- loads: W / Q-early / bias on the ACT sequencer (idle at start),
    # C / Q-late on SP; fp16 casts on DVE ---
    w_nat = singles.tile([128, HC, H], F32)  # [o in-chunk, om, h]
    nc.scalar.dma_start(out=w_nat[:],
                        in_=Wd.rearrange("(a p) h -> p a h", p=128))
    q_nat = cpool.tile([128, NQT, H], F32, tag="qnat", bufs=1)
    q_src = cpool.tile([128, NQT, H], LS, tag="qbf", bufs=1)
    q_view = Qd.rearrange("(a p) h -> p a h", p=128)
    nc.scalar.dma_start(out=q_nat[:, 0:4, :], in_=q_view[:, 0:4, :])
    bias = singles.tile([128, HC, 1], F32)
    nc.scalar.dma_start(out=bias[:, :, 0], in_=bd.rearrange("(c p) -> p c", p=128))
    nc.scalar.dma_start(out=q_nat[:, 4:8, :], in_=q_view[:, 4:8, :])

    w_bf = singles.tile([128, HC, H], LS, name="w_bf")
    nc.vector.tensor_copy(w_bf[:], w_nat[:])
    nc.vector.tensor_copy(q_src[:, 0:4, :], q_nat[:, 0:4, :])

    c_nats = {}
    c_bfs = {}

    def c_load(dc, cast=None):
        c_nats[dc] = cpool.tile([128, NDM, H], F32, tag="cnat", name=f"cnat{dc}")
        nc.sync.dma_start(
            out=c_nats[dc][:],
            in_=Cd[dc * DC:(dc + 1) * DC, :].rearrange("(a p) h -> p a h", p=128))
        c_bfs[dc] = cpool.tile([128, NDM, H], LS, tag="cbf", name=f"cbf{dc}")
        (cast or nc.gpsimd).tensor_copy(c_bfs[dc][:], c_nats[dc][:])

    c_load(0, cast=nc.vector)
    c_load(1)
    nc.sync.dma_start(out=q_nat[:, 8:12, :], in_=q_view[:, 8:12, :])
    nc.sync.dma_start(out=q_nat[:, 12:16, :], in_=q_view[:, 12:16, :])
    c_load(2)
    c_load(3)

    # qt[h, g, k, hc, q] = Q[g*512 + k*128 + q, hc*128 + h]
    # groups 0-1 on PE (latency-critical), 2-3 via XBAR
    qt = qstat.tile([128, NQG, 4, HC, 128], LS)

    def qt_pe(g):
        for hc in range(HC):
            pt = psum_t.tile([128, 512], LS, tag="pt", name=f"ptq{g}_{hc}")
            for k in range(4):
                nc.tensor.transpose(
                    pt[:, k * 128:(k + 1) * 128],
                    q_src[:, g * 4 + k, hc * 128:(hc + 1) * 128], ident[:])
            nc.vector.tensor_copy(
                qt[:, g, :, hc, :], pt.rearrange("p (a b) -> p a b", a=4))

    qt_pe(0)

    # --- W^T on PE: wt[h, hc, om*128+o] = W[om*128+o, hc*128+h] ---
    wt = qstat.tile([128, HC, H], LS)
    for om in range(HC):
        for hc in range(HC):
            pt = psum_t.tile([128, 128], LS, tag="pt")
            nc.tensor.transpose(pt[:], w_bf[:, om, hc * 128:(hc + 1) * 128], ident[:])
            nc.vector.tensor_copy(wt[:, hc, om * 128:(om + 1) * 128], pt[:])

    nc.vector.tensor_copy(q_src[:, 4:8, :], q_nat[:, 4:8, :])
    qt_pe(1)
    nc.vector.tensor_copy(q_src[:, 8:16, :], q_nat[:, 8:16, :])
    nc.sync.dma_start_transpose(qt[:, 2], q_src[:, 8:12, :])
    nc.sync.dma_start_transpose(qt[:, 3], q_src[:, 12:16, :])

    # --- C^T: ct[h, dm, hc, d] = C[dc*512 + dm*128 + d, hc*128 + h] ---
    cts = {}

    def c_prep(dc, on_pe=False):
        cts[dc] = ctpool.tile([128, NDM, HC, 128], LS, tag="ct", name=f"ct{dc}")
        if on_pe:
            for hc in range(HC):
                pt = psum_t.tile([128, 512], LS, tag="pt", name=f"ptc{dc}_{hc}")
                for dm in range(NDM):
                    nc.tensor.transpose(
                        pt[:, dm * 128:(dm + 1) * 128],
                        c_bfs[dc][:, dm, hc * 128:(hc + 1) * 128], ident[:])
                nc.vector.tensor_copy(
                    cts[dc][:, :, hc, :], pt.rearrange("p (a b) -> p a b", a=4))
        else:
            nc.sync.dma_start_transpose(cts[dc][:], c_bfs[dc][:])

    c_prep(0, on_pe=True)

    # --- per-group pipeline: linear+gelu -> QpT -> Qp ---
    qpt = qstat.tile([128, HC, QL], LS)
    qp = qstat.tile([128, NQT, H + 1], BF16)

    def q_group(qg):
        # linear + gelu for this 512-wide q block
        for om in range(HC):
            pl = psum_l.tile([128, 512], F32, tag="pl", name=f"plin{qg}_{om}")
            for hc in range(HC):
                nc.tensor.matmul(
                    pl[:],
                    wt[:, hc, om * 128:(om + 1) * 128],
                    qt[:, qg, :, hc, :],
                    start=(hc == 0),
                    stop=(hc == HC - 1),
                )
            nc.scalar.activation(
                qpt[:, om, qg * 512:(qg + 1) * 512], pl[:], AF.Gelu,
                bias=bias[:, om, :], scale=1.0,
            )
        # Qp natural for this group (PE back-transpose)
        for om in range(HC):
            pt = psum_t.tile([128, 512], LS, tag="pt", name=f"ptp{qg}_{om}")
            for k in range(4):
                qi = qg * 4 + k
                nc.tensor.transpose(
                    pt[:, k * 128:(k + 1) * 128],
                    qpt[:, om, qi * 128:(qi + 1) * 128], ident[:])
            nc.vector.tensor_copy(
                qp[:, qg * 4:(qg + 1) * 4, om * 128:(om + 1) * 128],
                pt.rearrange("p (a b) -> p a b", a=4))

    q_group(0)
    c_prep(1)
    q_group(1)
    q_group(2)
    q_group(3)
    nc.vector.memset(qp[:, :, H:H + 1], 1.0)

    # Lag the attended matmuls two q-tiles behind logits+exp so the PE
    # never waits on the ACT exp latency.
    LAG = 2
    for dc in range(NDC):
        c_nat = c_nats[dc]
        ct = cts[dc]
        expt = exppool.tile([128, NQT, DC], BF16)
        pa = [psum_a.tile([128, H + 1], F32, tag=f"a{dm}", name=f"pa{dm}")
              for dm in range(NDM)]
        for step in range(NQT + LAG):
            if step == 2 and dc >= 1 and dc + 3 < NDC:
                c_load(dc + 3)
            if step == 6 and dc + 2 < NDC and dc + 2 > 1:
                c_prep(dc + 2)
            if step < NQT:
                qi = step
                if qi in (5, 13) and dc > 0:
                    pl = psum_t.tile([128, DC], F32, tag="pt", name=f"plx{dc}_{qi}")
                else:
                    pl = psum_l.tile([128, DC], F32, tag="pl")
                for hc in range(HC):
                    nc.tensor.matmul(
                        pl[:],
                        qpt[:, hc, qi * 128:(qi + 1) * 128],
                        ct[:, :, hc, :],
                        start=(hc == 0),
                        stop=(hc == HC - 1),
                    )
                nc.scalar.activation(expt[:, qi, :], pl[:], AF.Exp)
            if step >= LAG:
                qj = step - LAG
                for dm in range(NDM):
                    nc.tensor.matmul(
                        pa[dm][:],
                        expt[:, qj, dm * 128:(dm + 1) * 128],
                        qp[:, qj, :],
                        start=(qj == 0),
                        stop=(qj == NQT - 1),
                    )

        o_sb = outpool.tile([128, NDM, H], F32)
        if dc == NDC - 1:
            halves = ((0, 1), (1, 2), (2, 3), (3, 4))
        else:
            halves = ((0, 4),)
        for lo, hi in halves:
            for dm in range(lo, hi):
                rec = small.tile([128, 1], F32)
                nc.vector.reciprocal(rec[:], pa[dm][:, H:H + 1])
                nc.vector.scalar_tensor_tensor(
                    o_sb[:, dm, :], pa[dm][:, 0:H], rec[:], c_nat[:, dm, :],
                    ALU.mult, ALU.add,
                )
            nc.sync.dma_start(
                out=Od[dc * DC + lo * 128:dc * DC + hi * 128, :]
                .rearrange("(a p) h -> p a h", p=128),
                in_=o_sb[:, lo:hi, :])
        del c_nats[dc], c_bfs[dc], cts[dc]


def build_nc():
    nc = bacc.Bacc("TRN2", target_bir_lowering=False, debug=False,
                   num_devices=N_CORES)
    Qd = nc.dram_tensor("Q", [QL, H], F32, kind="ExternalInput")
    Cd = nc.dram_tensor("C", [D, H], F32, kind="ExternalInput")
    Wd = nc.dram_tensor("W", [H, H], F32, kind="ExternalInput")
    bd = nc.dram_tensor("b", [H], F32, kind="ExternalInput")
    Od = nc.dram_tensor("out", [D, H], F32, kind="ExternalOutput")
    with tile.TileContext(nc) as tc:
        with ExitStack() as ctx:
            build_body(ctx, tc, nc, Qd[:], Cd[:], Wd[:], bd[:], Od[:])
    nc.finalize()
    return nc


_NC = None


def get_nc():
    global _NC
    if _NC is None:
        _NC = build_nc()
    return _NC


def kernel(Q, C, W, b):
    assert Q.shape == (B, QL, H) and C.shape == (B, D, H)
    nc = get_nc()
    in_maps = [
        {
            "Q": np.ascontiguousarray(Q[i], dtype=np.float32),
            "C": np.ascontiguousarray(C[i], dtype=np.float32),
            "W": np.ascontiguousarray(W, dtype=np.float32),
            "b": np.ascontiguousarray(b, dtype=np.float32),
        }
        for i in range(N_CORES)
    ]
    res = run_bass_kernel_spmd(nc, in_maps, core_ids=list(range(N_CORES)))
    return np.stack([res.results[i]["out"] for i in range(N_CORES)], axis=0)


# revision 13
# speedup vs baseline: 1.0192x; 1.0192x over previous
"""CQAttention Trainium2 kernel.

Math (per batch b, H=256, q=2048, d=8192):
  Qp   = gelu(Q @ W.T + b)                       [q, H]
  S    = C @ Qp.T                                [d, q]
  P    = softmax(S, axis=q)
  out  = P @ Qp + C                              [d, H]

Sharding: data-parallel over batch, one batch per NeuronCore (8 cores).

Per-core pipeline:
  - Loads are f32 HWDGE DMAs split across two sequencers (W/Q-early/bias
    on ACT which is idle at startup; C/Q-late plus all XBAR transposes
    and output stores on SP) with fp16 casts on DVE.
  - Latency-critical transposes at startup (W^T, Q^T groups 0-1, C^T
    chunk 0) run on the PE; the well-pipelined rest (Q^T groups 2-3, C^T
    chunks >= 1) on the DMA XBAR (dma_start_transpose, 14ns/16x128
    tile), keeping the PE free for matmuls.
  - QpT = gelu(W Q^T + b) with per-partition bias on ACT; Qp (natural,
    bf16) by PE-transposing QpT back, with a ones column so the softmax
    denominator falls out of the attended matmul's PSUM accumulation.
  - Per 512-row chunk of C: logits^T tiles [q=128, d=512] with fp16
    operands (bf16 fails the 2e-2 gate); exp on ACT straight from PSUM
    to bf16 (no max-subtraction: |logits| < ~70 so fp32 exp is safe);
    attended accumulated over 16 q-tiles into PSUM [d=128, 257] whose
    column 256 is the row-sum; fused epilogue
    out = (attended * 1/rowsum) + C in one DVE op per tile.
  - Chunk pipeline: C loads 3 chunks ahead, XBAR transposes 2 ahead,
    attended lags logits/exp by 2 q-tiles.
"""

from contextlib import ExitStack

import numpy as np

import concourse.mybir as mybir
import concourse.tile as tile
from concourse import bacc
from concourse.bass_utils import run_bass_kernel_spmd
from concourse.masks import make_identity

B, QL, D, H = 8, 2048, 8192, 256
N_CORES = 8
F32 = mybir.dt.float32
BF16 = mybir.dt.bfloat16
F16 = mybir.dt.float16

HC = H // 128      # feature chunks (2)
NQT = QL // 128    # q tiles (16)
NQG = NQT // 4     # q groups of 4 tiles (4)
DC = 512           # d-chunk size
NDC = D // DC      # d chunks (16)
NDM = DC // 128    # d tiles per chunk (4)

LS = F16  # logits-matmul operand dtype

AF = mybir.ActivationFunctionType
ALU = mybir.AluOpType


def build_body(ctx: ExitStack, tc: tile.TileContext, nc, Qd, Cd, Wd, bd, Od):
    singles = ctx.enter_context(tc.tile_pool(name="singles", bufs=1))
    qstat = ctx.enter_context(tc.tile_pool(name="qstat", bufs=1))
    cpool = ctx.enter_context(tc.tile_pool(name="cpool", bufs=5))
    ctpool = ctx.enter_context(tc.tile_pool(name="ctp", bufs=4))
    exppool = ctx.enter_context(tc.tile_pool(name="expp", bufs=2))
    outpool = ctx.enter_context(tc.tile_pool(name="outp", bufs=3))
    small = ctx.enter_context(tc.tile_pool(name="small", bufs=4))
    psum_l = ctx.enter_context(tc.tile_pool(name="psl", bufs=2, space="PSUM"))
    psum_t = ctx.enter_context(tc.tile_pool(name="pst", bufs=2, space="PSUM"))
    psum_a = ctx.enter_context(tc.tile_pool(name="psa", bufs=1, space="PSUM"))

    ident = singles.tile([128, 128], LS)
    make_identity(nc, ident)

    # Dummy matmuls to ramp the PE out of its p-state throttle (full clock
    # needs ~3us of continuous execution) while the first loads are in
    # flight; results are never read. A dummy gelu primes the ACT table
    # (1.28us load) during the same window.
    warm = singles.tile([128, 512], LS, name="warm")
    nc.gpsimd.memset(warm[:], 0.0)
    warm_o = singles.tile([128, 1], F32, name="warm_o")
    nc.scalar.activation(warm_o[:], warm[:, 0:1], AF.Gelu)
    for _ in range(12):
        pw = psum_l.tile([128, 512], F32, tag="pl", name="warmup")
        nc.tensor.matmul(pw[:], ident[:], warm[:], start=True, stop=True)

    # --- loads alternate between the two HWDGE queues (ACT idle at start,
    # SP) so the four Q transfers overlap; fp16 casts on DVE in arrival
    # order ---
    w_nat = singles.tile([128, HC, H], F32)  # [o in-chunk, om, h]
    nc.scalar.dma_start(out=w_nat[:],
                        in_=Wd.rearrange("(a p) h -> p a h", p=128))
    q_nat = cpool.tile([128, NQT, H], F32, tag="qnat", bufs=1)
    q_src = cpool.tile([128, NQT, H], LS, tag="qbf", bufs=1)
    q_view = Qd.rearrange("(a p) h -> p a h", p=128)
    nc.scalar.dma_start(out=q_nat[:, 0:4, :], in_=q_view[:, 0:4, :])

    c_nats = {}
    c_bfs = {}

    def c_load(dc, cast=None):
        c_nats[dc] = cpool.tile([128, NDM, H], F32, tag="cnat", name=f"cnat{dc}")
        nc.sync.dma_start(
            out=c_nats[dc][:],
            in_=Cd[dc * DC:(dc + 1) * DC, :].rearrange("(a p) h -> p a h", p=128))
        c_bfs[dc] = cpool.tile([128, NDM, H], LS, tag="cbf", name=f"cbf{dc}")
        (cast or nc.gpsimd).tensor_copy(c_bfs[dc][:], c_nats[dc][:])

    c_load(0, cast=nc.vector)
    nc.sync.dma_start(out=q_nat[:, 4:8, :], in_=q_view[:, 4:8, :])
    bias = singles.tile([128, HC, 1], F32)
    nc.scalar.dma_start(out=bias[:, :, 0], in_=bd.rearrange("(c p) -> p c", p=128))
    nc.scalar.dma_start(out=q_nat[:, 8:12, :], in_=q_view[:, 8:12, :])
    nc.sync.dma_start(out=q_nat[:, 12:16, :], in_=q_view[:, 12:16, :])
    c_load(1)
    c_load(2)
    c_load(3)

    w_bf = singles.tile([128, HC, H], LS, name="w_bf")
    nc.vector.tensor_copy(w_bf[:], w_nat[:])
    nc.vector.tensor_copy(q_src[:, 0:4, :], q_nat[:, 0:4, :])

    # qt[h, g, k, hc, q] = Q[g*512 + k*128 + q, hc*128 + h]
    # groups 0-1 on PE (latency-critical), 2-3 via XBAR
    qt = qstat.tile([128, NQG, 4, HC, 128], LS)

    def qt_pe(g):
        for hc in range(HC):
            pt = psum_t.tile([128, 512], LS, tag="pt", name=f"ptq{g}_{hc}")
            for k in range(4):
                nc.tensor.transpose(
                    pt[:, k * 128:(k + 1) * 128],
                    q_src[:, g * 4 + k, hc * 128:(hc + 1) * 128], ident[:])
            nc.vector.tensor_copy(
                qt[:, g, :, hc, :], pt.rearrange("p (a b) -> p a b", a=4))

    qt_pe(0)

    # --- W^T on PE: wt[h, hc, om*128+o] = W[om*128+o, hc*128+h] ---
    wt = qstat.tile([128, HC, H], LS)
    for om in range(HC):
        for hc in range(HC):
            pt = psum_t.tile([128, 128], LS, tag="pt")
            nc.tensor.transpose(pt[:], w_bf[:, om, hc * 128:(hc + 1) * 128], ident[:])
            nc.vector.tensor_copy(wt[:, hc, om * 128:(om + 1) * 128], pt[:])

    nc.vector.tensor_copy(q_src[:, 4:8, :], q_nat[:, 4:8, :])
    qt_pe(1)
    nc.vector.tensor_copy(q_src[:, 8:12, :], q_nat[:, 8:12, :])
    nc.sync.dma_start_transpose(qt[:, 2], q_src[:, 8:12, :])
    nc.vector.tensor_copy(q_src[:, 12:16, :], q_nat[:, 12:16, :])
    nc.sync.dma_start_transpose(qt[:, 3], q_src[:, 12:16, :])

    # --- C^T: ct[h, dm, hc, d] = C[dc*512 + dm*128 + d, hc*128 + h] ---
    cts = {}

    def c_prep(dc, on_pe=False):
        cts[dc] = ctpool.tile([128, NDM, HC, 128], LS, tag="ct", name=f"ct{dc}")
        if on_pe:
            for hc in range(HC):
                pt = psum_t.tile([128, 512], LS, tag="pt", name=f"ptc{dc}_{hc}")
                for dm in range(NDM):
                    nc.tensor.transpose(
                        pt[:, dm * 128:(dm + 1) * 128],
                        c_bfs[dc][:, dm, hc * 128:(hc + 1) * 128], ident[:])
                nc.vector.tensor_copy(
                    cts[dc][:, :, hc, :], pt.rearrange("p (a b) -> p a b", a=4))
        else:
            nc.sync.dma_start_transpose(cts[dc][:], c_bfs[dc][:])

    c_prep(0, on_pe=True)

    # --- per-group pipeline: linear+gelu -> QpT -> Qp ---
    qpt = qstat.tile([128, HC, QL], LS)
    qp = qstat.tile([128, NQT, H + 1], BF16)

    def q_group(qg):
        # linear + gelu for this 512-wide q block
        for om in range(HC):
            pl = psum_l.tile([128, 512], F32, tag="pl", name=f"plin{qg}_{om}")
            for hc in range(HC):
                nc.tensor.matmul(
                    pl[:],
                    wt[:, hc, om * 128:(om + 1) * 128],
                    qt[:, qg, :, hc, :],
                    start=(hc == 0),
                    stop=(hc == HC - 1),
                )
            nc.scalar.activation(
                qpt[:, om, qg * 512:(qg + 1) * 512], pl[:], AF.Gelu,
                bias=bias[:, om, :], scale=1.0,
            )
        # Qp natural for this group (PE back-transpose)
        for om in range(HC):
            pt = psum_t.tile([128, 512], LS, tag="pt", name=f"ptp{qg}_{om}")
            for k in range(4):
                qi = qg * 4 + k
                nc.tensor.transpose(
                    pt[:, k * 128:(k + 1) * 128],
                    qpt[:, om, qi * 128:(qi + 1) * 128], ident[:])
            nc.vector.tensor_copy(
                qp[:, qg * 4:(qg + 1) * 4, om * 128:(om + 1) * 128],
                pt.rearrange("p (a b) -> p a b", a=4))

    q_group(0)
    c_prep(1)
    q_group(1)
    q_group(2)
    q_group(3)
    nc.vector.memset(qp[:, :, H:H + 1], 1.0)

    # Lag the attended matmuls two q-tiles behind logits+exp so the PE
    # never waits on the ACT exp latency.
    LAG = 2
    for dc in range(NDC):
        c_nat = c_nats[dc]
        ct = cts[dc]
        expt = exppool.tile([128, NQT, DC], BF16)
        pa = [psum_a.tile([128, H + 1], F32, tag=f"a{dm}", name=f"pa{dm}")
              for dm in range(NDM)]
        for step in range(NQT + LAG):
            if step == 2 and dc >= 1 and dc + 3 < NDC:
                c_load(dc + 3)
            if step == 6 and dc + 2 < NDC and dc + 2 > 1:
                c_prep(dc + 2)
            if step < NQT:
                qi = step
                if qi in (5, 13) and dc > 0:
                    pl = psum_t.tile([128, DC], F32, tag="pt", name=f"plx{dc}_{qi}")
                else:
                    pl = psum_l.tile([128, DC], F32, tag="pl")
                for hc in range(HC):
                    nc.tensor.matmul(
                        pl[:],
                        qpt[:, hc, qi * 128:(qi + 1) * 128],
                        ct[:, :, hc, :],
                        start=(hc == 0),
                        stop=(hc == HC - 1),
                    )
                nc.scalar.activation(expt[:, qi, :], pl[:], AF.Exp)
            if step >= LAG:
                qj = step - LAG
                for dm in range(NDM):
                    nc.tensor.matmul(
                        pa[dm][:],
                        expt[:, qj, dm * 128:(dm + 1) * 128],
                        qp[:, qj, :],
                        start=(qj == 0),
                        stop=(qj == NQT - 1),
                    )

        o_sb = outpool.tile([128, NDM, H], F32)
        if dc == NDC - 1:
            halves = ((0, 1), (1, 2), (2, 3), (3, 4))
        else:
            halves = ((0, 4),)
        for lo, hi in halves:
            for dm in range(lo, hi):
                rec = small.tile([128, 1], F32)
                nc.vector.reciprocal(rec[:], pa[dm][:, H:H + 1])
                nc.vector.scalar_tensor_tensor(
                    o_sb[:, dm, :], pa[dm][:, 0:H], rec[:], c_nat[:, dm, :],
                    ALU.mult, ALU.add,
                )
            nc.sync.dma_start(
                out=Od[dc * DC + lo * 128:dc * DC + hi * 128, :]
                .rearrange("(a p) h -> p a h", p=128),
                in_=o_sb[:, lo:hi, :])
        del c_nats[dc], c_bfs[dc], cts[dc]


def build_nc():
    nc = bacc.Bacc("TRN2", target_bir_lowering=False, debug=False,
                   num_devices=N_CORES)
    Qd = nc.dram_tensor("Q", [QL, H], F32, kind="ExternalInput")
    Cd = nc.dram_tensor("C", [D, H], F32, kind="ExternalInput")
    Wd = nc.dram_tensor("W", [H, H], F32, kind="ExternalInput")
    bd = nc.dram_tensor("b", [H], F32, kind="ExternalInput")
    Od = nc.dram_tensor("out", [D, H], F32, kind="ExternalOutput")
    with tile.TileContext(nc) as tc:
        with ExitStack() as ctx:
            build_body(ctx, tc, nc, Qd[:], Cd[:], Wd[:], bd[:], Od[:])
    nc.finalize()
    return nc


_NC = None


def get_nc():
    global _NC
    if _NC is None:
        _NC = build_nc()
    return _NC


def kernel(Q, C, W, b):
    assert Q.shape == (B, QL, H) and C.shape == (B, D, H)
    nc = get_nc()
    in_maps = [
        {
            "Q": np.ascontiguousarray(Q[i], dtype=np.float32),
            "C": np.ascontiguousarray(C[i], dtype=np.float32),
            "W": np.ascontiguousarray(W, dtype=np.float32),
            "b": np.ascontiguousarray(b, dtype=np.float32),
        }
        for i in range(N_CORES)
    ]
    res = run_bass_kernel_spmd(nc, in_maps, core_ids=list(range(N_CORES)))
    return np.stack([res.results[i]["out"] for i in range(N_CORES)], axis=0)


# revision 14
# speedup vs baseline: 1.0210x; 1.0018x over previous
"""CQAttention Trainium2 kernel.

Math (per batch b, H=256, q=2048, d=8192):
  Qp   = gelu(Q @ W.T + b)                       [q, H]
  S    = C @ Qp.T                                [d, q]
  P    = softmax(S, axis=q)
  out  = P @ Qp + C                              [d, H]

Sharding: data-parallel over batch, one batch per NeuronCore (8 cores).

Per-core pipeline:
  - Loads are f32 HWDGE DMAs split across two sequencers (W/Q-early/bias
    on ACT which is idle at startup; C/Q-late plus all XBAR transposes
    and output stores on SP) with fp16 casts on DVE.
  - Latency-critical transposes at startup (W^T, Q^T groups 0-1, C^T
    chunk 0) run on the PE; the well-pipelined rest (Q^T groups 2-3, C^T
    chunks >= 1) on the DMA XBAR (dma_start_transpose, 14ns/16x128
    tile), keeping the PE free for matmuls.
  - QpT = gelu(W Q^T + b) with per-partition bias on ACT; Qp (natural,
    bf16) by PE-transposing QpT back, with a ones column so the softmax
    denominator falls out of the attended matmul's PSUM accumulation.
  - Per 512-row chunk of C: logits^T tiles [q=128, d=512] with fp16
    operands (bf16 fails the 2e-2 gate); exp on ACT straight from PSUM
    to bf16 (no max-subtraction: |logits| < ~70 so fp32 exp is safe);
    attended accumulated over 16 q-tiles into PSUM [d=128, 257] whose
    column 256 is the row-sum; fused epilogue
    out = (attended * 1/rowsum) + C in one DVE op per tile.
  - Chunk pipeline: C loads 3 chunks ahead, XBAR transposes 2 ahead,
    attended lags logits/exp by 2 q-tiles.
"""

from contextlib import ExitStack

import numpy as np

import concourse.mybir as mybir
import concourse.tile as tile
from concourse import bacc
from concourse.bass_utils import run_bass_kernel_spmd
from concourse.masks import make_identity

B, QL, D, H = 8, 2048, 8192, 256
N_CORES = 8
F32 = mybir.dt.float32
BF16 = mybir.dt.bfloat16
F16 = mybir.dt.float16

HC = H // 128      # feature chunks (2)
NQT = QL // 128    # q tiles (16)
NQG = NQT // 4     # q groups of 4 tiles (4)
DC = 512           # d-chunk size
NDC = D // DC      # d chunks (16)
NDM = DC // 128    # d tiles per chunk (4)

LS = F16  # logits-matmul operand dtype

AF = mybir.ActivationFunctionType
ALU = mybir.AluOpType


def build_body(ctx: ExitStack, tc: tile.TileContext, nc, Qd, Cd, Wd, bd, Od):
    singles = ctx.enter_context(tc.tile_pool(name="singles", bufs=1))
    qstat = ctx.enter_context(tc.tile_pool(name="qstat", bufs=1))
    cpool = ctx.enter_context(tc.tile_pool(name="cpool", bufs=5))
    ctpool = ctx.enter_context(tc.tile_pool(name="ctp", bufs=4))
    exppool = ctx.enter_context(tc.tile_pool(name="expp", bufs=2))
    outpool = ctx.enter_context(tc.tile_pool(name="outp", bufs=3))
    small = ctx.enter_context(tc.tile_pool(name="small", bufs=4))
    psum_l = ctx.enter_context(tc.tile_pool(name="psl", bufs=2, space="PSUM"))
    psum_t = ctx.enter_context(tc.tile_pool(name="pst", bufs=2, space="PSUM"))
    psum_a = ctx.enter_context(tc.tile_pool(name="psa", bufs=1, space="PSUM"))

    ident = singles.tile([128, 128], LS)
    make_identity(nc, ident)

    # Dummy matmuls to ramp the PE out of its p-state throttle (full clock
    # needs ~3us of continuous execution) while the first loads are in
    # flight; results are never read. A dummy gelu primes the ACT table
    # (1.28us load) during the same window.
    warm = singles.tile([128, 512], LS, name="warm")
    nc.gpsimd.memset(warm[:], 0.0)
    warm_o = singles.tile([128, 1], F32, name="warm_o")
    nc.scalar.activation(warm_o[:], warm[:, 0:1], AF.Gelu)
    for _ in range(12):
        pw = psum_l.tile([128, 512], F32, tag="pl", name="warmup")
        nc.tensor.matmul(pw[:], ident[:], warm[:], start=True, stop=True)

    # --- loads alternate between the two HWDGE queues (ACT idle at start,
    # SP) so the four Q transfers overlap; fp16 casts on DVE in arrival
    # order ---
    w_nat = singles.tile([128, HC, H], F32)  # [o in-chunk, om, h]
    nc.scalar.dma_start(out=w_nat[:],
                        in_=Wd.rearrange("(a p) h -> p a h", p=128))
    q_nat = cpool.tile([128, NQT, H], F32, tag="qnat", bufs=1)
    q_src = cpool.tile([128, NQT, H], LS, tag="qbf", bufs=1)
    q_view = Qd.rearrange("(p a) h -> p a h", p=128)
    nc.scalar.dma_start(out=q_nat[:, 0:4, :], in_=q_view[:, 0:4, :])

    c_nats = {}
    c_bfs = {}

    def c_load(dc, cast=None):
        c_nats[dc] = cpool.tile([128, NDM, H], F32, tag="cnat", name=f"cnat{dc}")
        nc.sync.dma_start(
            out=c_nats[dc][:],
            in_=Cd[dc * DC:(dc + 1) * DC, :].rearrange("(p a) h -> p a h", p=128))
        c_bfs[dc] = cpool.tile([128, NDM, H], LS, tag="cbf", name=f"cbf{dc}")
        (cast or nc.gpsimd).tensor_copy(c_bfs[dc][:], c_nats[dc][:])

    c_load(0, cast=nc.vector)
    nc.sync.dma_start(out=q_nat[:, 4:8, :], in_=q_view[:, 4:8, :])
    bias = singles.tile([128, HC, 1], F32)
    nc.scalar.dma_start(out=bias[:, :, 0], in_=bd.rearrange("(c p) -> p c", p=128))
    nc.scalar.dma_start(out=q_nat[:, 8:12, :], in_=q_view[:, 8:12, :])
    nc.sync.dma_start(out=q_nat[:, 12:16, :], in_=q_view[:, 12:16, :])
    c_load(1)
    c_load(2)
    c_load(3)

    w_bf = singles.tile([128, HC, H], LS, name="w_bf")
    nc.vector.tensor_copy(w_bf[:], w_nat[:])
    nc.vector.tensor_copy(q_src[:, 0:4, :], q_nat[:, 0:4, :])

    # qt[h, g, k, hc, q] = Q[g*512 + k*128 + q, hc*128 + h]
    # groups 0-1 on PE (latency-critical), 2-3 via XBAR
    qt = qstat.tile([128, NQG, 4, HC, 128], LS)

    def qt_pe(g):
        for hc in range(HC):
            pt = psum_t.tile([128, 512], LS, tag="pt", name=f"ptq{g}_{hc}")
            for k in range(4):
                nc.tensor.transpose(
                    pt[:, k * 128:(k + 1) * 128],
                    q_src[:, g * 4 + k, hc * 128:(hc + 1) * 128], ident[:])
            nc.vector.tensor_copy(
                qt[:, g, :, hc, :], pt.rearrange("p (a b) -> p a b", a=4))

    qt_pe(0)

    # --- W^T on PE: wt[h, hc, om*128+o] = W[om*128+o, hc*128+h] ---
    wt = qstat.tile([128, HC, H], LS)
    for om in range(HC):
        for hc in range(HC):
            pt = psum_t.tile([128, 128], LS, tag="pt")
            nc.tensor.transpose(pt[:], w_bf[:, om, hc * 128:(hc + 1) * 128], ident[:])
            nc.vector.tensor_copy(wt[:, hc, om * 128:(om + 1) * 128], pt[:])

    nc.vector.tensor_copy(q_src[:, 4:8, :], q_nat[:, 4:8, :])
    qt_pe(1)
    nc.vector.tensor_copy(q_src[:, 8:12, :], q_nat[:, 8:12, :])
    nc.sync.dma_start_transpose(qt[:, 2], q_src[:, 8:12, :])
    nc.vector.tensor_copy(q_src[:, 12:16, :], q_nat[:, 12:16, :])
    nc.sync.dma_start_transpose(qt[:, 3], q_src[:, 12:16, :])

    # --- C^T: ct[h, dm, hc, d] = C[dc*512 + dm*128 + d, hc*128 + h] ---
    cts = {}

    def c_prep(dc, on_pe=False):
        cts[dc] = ctpool.tile([128, NDM, HC, 128], LS, tag="ct", name=f"ct{dc}")
        if on_pe:
            for hc in range(HC):
                pt = psum_t.tile([128, 512], LS, tag="pt", name=f"ptc{dc}_{hc}")
                for dm in range(NDM):
                    nc.tensor.transpose(
                        pt[:, dm * 128:(dm + 1) * 128],
                        c_bfs[dc][:, dm, hc * 128:(hc + 1) * 128], ident[:])
                nc.vector.tensor_copy(
                    cts[dc][:, :, hc, :], pt.rearrange("p (a b) -> p a b", a=4))
        else:
            nc.sync.dma_start_transpose(cts[dc][:], c_bfs[dc][:])

    c_prep(0, on_pe=True)

    # --- per-group pipeline: linear+gelu -> QpT -> Qp ---
    qpt = qstat.tile([128, HC, QL], LS)
    qp = qstat.tile([128, NQT, H + 1], BF16)

    def q_group(qg):
        # linear + gelu for this 512-wide q block
        for om in range(HC):
            pl = psum_l.tile([128, 512], F32, tag="pl", name=f"plin{qg}_{om}")
            for hc in range(HC):
                nc.tensor.matmul(
                    pl[:],
                    wt[:, hc, om * 128:(om + 1) * 128],
                    qt[:, qg, :, hc, :],
                    start=(hc == 0),
                    stop=(hc == HC - 1),
                )
            nc.scalar.activation(
                qpt[:, om, qg * 512:(qg + 1) * 512], pl[:], AF.Gelu,
                bias=bias[:, om, :], scale=1.0,
            )
        # Qp natural for this group (PE back-transpose)
        for om in range(HC):
            pt = psum_t.tile([128, 512], LS, tag="pt", name=f"ptp{qg}_{om}")
            for k in range(4):
                qi = qg * 4 + k
                nc.tensor.transpose(
                    pt[:, k * 128:(k + 1) * 128],
                    qpt[:, om, qi * 128:(qi + 1) * 128], ident[:])
            nc.vector.tensor_copy(
                qp[:, qg * 4:(qg + 1) * 4, om * 128:(om + 1) * 128],
                pt.rearrange("p (a b) -> p a b", a=4))

    q_group(0)
    c_prep(1)
    q_group(1)
    q_group(2)
    q_group(3)
    nc.vector.memset(qp[:, :, H:H + 1], 1.0)

    # Lag the attended matmuls two q-tiles behind logits+exp so the PE
    # never waits on the ACT exp latency.
    LAG = 2
    for dc in range(NDC):
        c_nat = c_nats[dc]
        ct = cts[dc]
        expt = exppool.tile([128, NQT, DC], BF16)
        pa = [psum_a.tile([128, H + 1], F32, tag=f"a{dm}", name=f"pa{dm}")
              for dm in range(NDM)]
        for step in range(NQT + LAG):
            if step == 2 and dc >= 1 and dc + 3 < NDC:
                c_load(dc + 3)
            if step == 6 and dc + 2 < NDC and dc + 2 > 1:
                c_prep(dc + 2)
            if step < NQT:
                qi = step
                if qi in (5, 13) and dc > 0:
                    pl = psum_t.tile([128, DC], F32, tag="pt", name=f"plx{dc}_{qi}")
                else:
                    pl = psum_l.tile([128, DC], F32, tag="pl")
                for hc in range(HC):
                    nc.tensor.matmul(
                        pl[:],
                        qpt[:, hc, qi * 128:(qi + 1) * 128],
                        ct[:, :, hc, :],
                        start=(hc == 0),
                        stop=(hc == HC - 1),
                    )
                nc.scalar.activation(expt[:, qi, :], pl[:], AF.Exp)
            if step >= LAG:
                qj = step - LAG
                for dm in range(NDM):
                    nc.tensor.matmul(
                        pa[dm][:],
                        expt[:, qj, dm * 128:(dm + 1) * 128],
                        qp[:, qj, :],
                        start=(qj == 0),
                        stop=(qj == NQT - 1),
                    )

        o_sb = outpool.tile([128, NDM, H], F32)
        if dc == NDC - 1:
            halves = ((0, 1), (1, 2), (2, 3), (3, 4))
        else:
            halves = ((0, 4),)
        for lo, hi in halves:
            for dm in range(lo, hi):
                rec = small.tile([128, 1], F32)
                nc.vector.reciprocal(rec[:], pa[dm][:, H:H + 1])
                nc.vector.scalar_tensor_tensor(
                    o_sb[:, dm, :], pa[dm][:, 0:H], rec[:], c_nat[:, dm, :],
                    ALU.mult, ALU.add,
                )
            nc.sync.dma_start(
                out=Od[dc * DC:(dc + 1) * DC, :]
                .rearrange("(p a) h -> p a h", p=128)[:, lo:hi, :],
                in_=o_sb[:, lo:hi, :])
        del c_nats[dc], c_bfs[dc], cts[dc]


def build_nc():
    nc = bacc.Bacc("TRN2", target_bir_lowering=False, debug=False,
                   num_devices=N_CORES)
    Qd = nc.dram_tensor("Q", [QL, H], F32, kind="ExternalInput")
    Cd = nc.dram_tensor("C", [D, H], F32, kind="ExternalInput")
    Wd = nc.dram_tensor("W", [H, H], F32, kind="ExternalInput")
    bd = nc.dram_tensor("b", [H], F32, kind="ExternalInput")
    Od = nc.dram_tensor("out", [D, H], F32, kind="ExternalOutput")
    with tile.TileContext(nc) as tc:
        with ExitStack() as ctx:
            build_body(ctx, tc, nc, Qd[:], Cd[:], Wd[:], bd[:], Od[:])
    nc.finalize()
    return nc


_NC = None


def get_nc():
    global _NC
    if _NC is None:
        _NC = build_nc()
    return _NC


def kernel(Q, C, W, b):
    assert Q.shape == (B, QL, H) and C.shape == (B, D, H)
    nc = get_nc()
    in_maps = [
        {
            "Q": np.ascontiguousarray(Q[i], dtype=np.float32),
            "C": np.ascontiguousarray(C[i], dtype=np.float32),
            "W": np.ascontiguousarray(W, dtype=np.float32),
            "b": np.ascontiguousarray(b, dtype=np.float32),
        }
        for i in range(N_CORES)
    ]
    res = run_bass_kernel_spmd(nc, in_maps, core_ids=list(range(N_CORES)))
    return np.stack([res.results[i]["out"] for i in range(N_CORES)], axis=0)
